# revision 23
# baseline (speedup 1.0000x reference)
"""AdaptiveTokenSampling on 8 TRN2 NeuronCores (Bass/Tile, batch-parallel).

Per-core (one batch element):
  1. score pipeline: value norms + cls attention -> pseudo-logits (token-partition layout)
  2. gumbel argmax sampling (vector.max/max_index) -> 256 sampled token ids
  3. sort-based unique via comparison matrices (DVE) + positional scatter (PE matmuls)
  4. row gather of attn via dma_gather across 4 SWDGE queues; writes on both HWDGE queues

The attn input is host-padded to 1088-float rows (4352 B, a multiple of 256) so the
custom-ISA dma_gather row stride constraint holds.
"""
import numpy as np

import concourse.bacc as bacc
import concourse.bass as bass
import concourse.mybir as mybir
import concourse.tile as tile
from concourse.bass_utils import run_bass_kernel_spmd

F32 = mybir.dt.float32
I32 = mybir.dt.int32
I16 = mybir.dt.int16
U32 = mybir.dt.uint32
U8 = mybir.dt.uint8

B, H, N, D, K = 8, 12, 1025, 64, 256
W = 1088             # padded attn row length (f32), 4352 B stride for dma_gather
NM1 = N - 1          # 1024
NH = N * H           # 12300 rows in padded attn table
KP1 = K + 1          # 257
EPS = 1e-6
MASKVAL = float(-np.finfo(np.float32).max / 2)
DUMP = 300.0         # parking slot for non-survivors (contributes 0 everywhere)
VCH = 3              # heads per value-pipeline chunk
ALU = mybir.AluOpType


def _build():
    nc = bacc.Bacc(None, target_bir_lowering=False, debug=False, num_devices=8,
                   num_swdge_queues=4)

    attn_d = nc.declare_dram_parameter("attn", [NH, W], F32, isOutput=False)
    val_d = nc.declare_dram_parameter("value", [128, H * 512], F32, isOutput=False)
    cls_d = nc.declare_dram_parameter("clsp", [128, H * 8], F32, isOutput=False)
    msk_d = nc.declare_dram_parameter("maskp", [N], U8, isOutput=False)
    gum_d = nc.declare_dram_parameter("gumbel", [K, NM1], F32, isOutput=False)

    oattn_d = nc.declare_dram_parameter("out_attn", [H, KP1, W], F32, isOutput=True)
    oids_d = nc.declare_dram_parameter("out_ids", [KP1], I32, isOutput=True)
    omask_d = nc.declare_dram_parameter("out_mask", [KP1], U8, isOutput=True)

    pl_dram = nc.dram_tensor("pl_dram", [NM1], F32)

    ident_c = nc.inline_tensor(np.eye(128, dtype=np.float32), name="ident_c")
    iota128_c = nc.inline_tensor(
        np.broadcast_to(np.arange(128, dtype=np.float32), (128, 128)).copy(), name="iota128_c")
    iota16r_c = nc.inline_tensor(
        np.broadcast_to(np.arange(128, dtype=np.float32) % 16, (128, 128)).copy(),
        name="iota16r_c")  # [p, x] = x % 16 (same every row)
    iota16f_c = nc.inline_tensor(
        np.broadcast_to(np.arange(16, dtype=np.float32), (128, 16)).copy(), name="iota16f_c")
    x128 = np.arange(128, dtype=np.float32)
    mod16_c = nc.inline_tensor(np.broadcast_to(x128 % 16, (128, 128)).copy(), name="mod16_c")
    div16_c = nc.inline_tensor(np.broadcast_to(x128 // 16, (128, 128)).copy(), name="div16_c")
    gidx = np.arange(K, dtype=np.float32)
    p128 = np.arange(128, dtype=np.float32)
    # ILTrev_a[p, q] = 1.0 if q < (p+128a): count of earlier-equal, target on partition
    iltr_c = [nc.inline_tensor((gidx[None, :] < (p128 + 128 * a)[:, None]).astype(np.float32),
                               name=f"iltr{a}_c") for a in range(2)]
    # head offsets for wrapped gather indices: [128, 16h+f] -> h*N
    hoffw_c = nc.inline_tensor(
        np.broadcast_to(np.repeat(np.arange(H, dtype=np.float32) * float(N), 16), (128, 16 * H)).copy(),
        name="hoffw_c")
    dummyi_c = nc.inline_tensor(np.zeros((128, 1), dtype=np.int16), name="dummyi_c")

    with tile.TileContext(nc) as tc:
        with (
            tc.tile_pool(name="const", bufs=1) as cp,
            tc.tile_pool(name="work", bufs=2) as wp,
            tc.tile_pool(name="keep", bufs=1) as kp,
            tc.tile_pool(name="ps", bufs=2, space="PSUM") as ps,
            tc.tile_pool(name="psacc", bufs=2, space="PSUM") as psa,
            tc.tile_pool(name="gath", bufs=16) as gp,
        ):
            # ---- tiny DVE constants + ACT table warmups (off critical path) ----
            ones_col = cp.tile([128, 1], F32)
            nc.vector.memset(ones_col[:], 1.0)
            ones_row = cp.tile([1, 128], F32)
            nc.vector.memset(ones_row[:], 1.0)
            eps_col = cp.tile([128, 1], F32)
            nc.vector.memset(eps_col[:], EPS)
            zero_i = cp.tile([1, 1], I32)
            nc.vector.memset(zero_i[:], 0)
            one_u8 = cp.tile([1, 1], U8)
            nc.vector.memset(one_u8[:], 1)
            dumm = wp.tile([1, 1], F32, tag="dumm")
            nc.scalar.square(dumm[:], ones_col[0:1, 0:1])
            nc.scalar.activation(dumm[:], ones_col[0:1, 0:1], mybir.ActivationFunctionType.Sqrt)
            nc.scalar.activation(dumm[:], ones_col[0:1, 0:1], mybir.ActivationFunctionType.Ln,
                                 bias=eps_col[0:1, 0:1], scale=1.0)

            # ---- gpsimd library warmup: tiny dma_gather so the Q7 IRAM load happens early ----
            dzi = cp.tile([128, 1], I16)
            nc.scalar.dma_start(out=dzi[:], in_=dummyi_c[:])
            dg = gp.tile([128, W], F32, tag="g")
            nc.gpsimd.dma_gather(
                out_ap=dg[:].rearrange("p (t f) -> p t f", f=W),
                in_ap=attn_d[:], idxs_ap=dzi[:], num_idxs=16, num_idxs_reg=16,
                elem_size=W, queue_num=0,
            )


            # ---- input loads first: value chunks (both queues) + gumbel ----
            vts = []
            for k in range(H // VCH):
                vt = kp.tile([128, VCH * 512], F32, tag=f"vt{k}")
                (nc.sync if k % 2 == 0 else nc.scalar).dma_start(
                    out=vt[:], in_=val_d[:][:, k * VCH * 512:(k + 1) * VCH * 512])
                vts.append(vt)
            gts = []
            for a in range(2):
                gt = kp.tile([128, NM1], F32, tag=f"gt{a}")
                (nc.sync if a == 0 else nc.scalar).dma_start(
                    out=gt[:], in_=gum_d[:][a * 128:(a + 1) * 128, :])
                gts.append(gt)

            # ---- static work with no deps: cls row of new_attn, ids[0], mask[0] ----
            g0 = cp.tile([H, W], F32, tag="g0")
            nc.scalar.dma_start(out=g0[:], in_=bass.AP(attn_d, 0, [[N * W, H], [1, W]]))
            nc.scalar.dma_start(out=oattn_d[:][:, 0, :], in_=g0[:], single_packet=True)

            # ---- constants (scalar queue; value pipeline owns the sync queue) ----
            ident = cp.tile([128, 128], F32)
            nc.scalar.dma_start(out=ident[:], in_=ident_c[:])
            iota128 = cp.tile([128, 128], F32)
            nc.scalar.dma_start(out=iota128[:], in_=iota128_c[:])
            iota16r = cp.tile([128, 128], F32)
            nc.scalar.dma_start(out=iota16r[:], in_=iota16r_c[:])
            iota16f = cp.tile([128, 16], F32)
            nc.scalar.dma_start(out=iota16f[:], in_=iota16f_c[:])
            mod16t = cp.tile([128, 128], F32)
            nc.scalar.dma_start(out=mod16t[:], in_=mod16_c[:])
            div16t = cp.tile([128, 128], F32)
            nc.scalar.dma_start(out=div16t[:], in_=div16_c[:])
            iltr = []
            for a in range(2):
                t2 = cp.tile([128, K], F32, tag=f"iltr{a}")
                nc.scalar.dma_start(out=t2[:], in_=iltr_c[a][:])
                iltr.append(t2)
            hoffw = cp.tile([128, 16 * H], F32)
            nc.scalar.dma_start(out=hoffw[:], in_=hoffw_c[:])
            cls = kp.tile([128, H * 8], F32)
            nc.gpsimd.dma_start(out=cls[:], in_=cls_d[:])
            mku = wp.tile([128, 8], U8)
            nc.gpsimd.dma_start(out=mku[:], in_=msk_d[:][None, 1:].rearrange("o (p c) -> (o p) c", c=8))

            # PE warmup: observe const DMAs once so PE-transposes need only a DVE wait
            warm = ps.tile([1, 1], F32, tag="small")
            nc.tensor.matmul(warm[:], lhsT=ident[:, 0:1], rhs=iota128[:, 0:1],
                             start=True, stop=True)

            # ---- stage 1: scores (token-partition layout: token j-1 = 8p + c) ----
            norms2 = kp.tile([128, H * 8], F32)
            for k in range(H // VCH):
                h0 = k * VCH
                sqc = wp.tile([128, VCH * 512], F32, tag="sqc")
                nc.scalar.square(sqc[:], vts[k][:])
                nc.vector.tensor_reduce(
                    out=norms2[:, h0 * 8:(h0 + VCH) * 8],
                    in_=sqc[:].rearrange("p (g d) -> p g d", d=D),
                    axis=mybir.AxisListType.X, op=ALU.add)
            norms = kp.tile([128, H * 8], F32)
            nc.scalar.sqrt(norms[:], norms2[:])

            prod = kp.tile([128, H * 8], F32)
            nc.vector.tensor_mul(prod[:], cls[:], norms[:])
            score = kp.tile([128, 8], F32)
            nc.vector.tensor_reduce(
                out=score[:], in_=prod[:].rearrange("p (h c) -> p c h", c=8),
                axis=mybir.AxisListType.X, op=ALU.add)

            sumrow = wp.tile([128, 1], F32)
            nc.vector.tensor_reduce(out=sumrow[:], in_=score[:],
                                    axis=mybir.AxisListType.X, op=ALU.add)
            total_ps = ps.tile([1, 1], F32, tag="small")
            nc.tensor.matmul(total_ps[:], lhsT=sumrow[:], rhs=ones_col[:],
                             start=True, stop=True)
            total = wp.tile([1, 1], F32)
            nc.vector.tensor_scalar(total[:], total_ps[:], EPS, None, op0=ALU.add)
            recip = wp.tile([1, 1], F32)
            nc.vector.reciprocal(recip[:], total[:])
            # broadcast recip to 128 partitions: K=1 matmul with a ones row (exact: 1.0*x)
            recB_ps = ps.tile([128, 1], F32, tag="small")
            nc.tensor.matmul(recB_ps[:], lhsT=ones_row[:], rhs=recip[:],
                             start=True, stop=True)
            recipB = wp.tile([128, 1], F32)
            nc.vector.tensor_copy(recipB[:], recB_ps[:])

            pl = kp.tile([128, 8], F32)
            nc.scalar.activation(pl[:], score[:], mybir.ActivationFunctionType.Ln,
                                 bias=eps_col[:, 0:1], scale=recipB[:, 0:1])
            # mask (all ones in practice; exact reference semantics)
            mkf = wp.tile([128, 8], F32)
            nc.vector.tensor_copy(mkf[:], mku[:])
            plm = kp.tile([128, 8], F32)
            nc.vector.tensor_mul(plm[:], pl[:], mkf[:])
            inv = wp.tile([128, 8], F32)
            nc.vector.tensor_scalar(inv[:], mkf[:], 0.5, MASKVAL, op0=ALU.is_lt, op1=ALU.mult)
            nc.vector.tensor_add(plm[:], plm[:], inv[:])
            nc.sync.dma_start(out=pl_dram[:].rearrange("(p c) -> p c", c=8), in_=plm[:])
            # replicate pl to all 128 partitions (exact bytes via DRAM replicate read)
            plB = kp.tile([128, NM1], F32)
            nc.sync.dma_start(out=plB[:], in_=bass.AP(pl_dram, 0, [[0, 128], [1, NM1]]))

            # ---- stage 2: gumbel argmax sampling ----
            ids_col = []
            for a in range(2):
                lg = wp.tile([128, NM1], F32, tag="lg")
                nc.vector.tensor_add(lg[:], gts[a][:], plB[:])
                mx8 = wp.tile([128, 8], F32, tag="mx8")
                nc.vector.max(out=mx8[:], in_=lg[:])
                ix8 = wp.tile([128, 8], U32, tag="ix8")
                nc.vector.max_index(out=ix8[:], in_max=mx8[:], in_values=lg[:])
                idc = kp.tile([128, 1], F32, tag=f"idc{a}")
                nc.vector.tensor_scalar(idc[:], ix8[:, 0:1], 1.0, None, op0=ALU.add)
                ids_col.append(idc)

            # ---- stage 3: unique + sorted positions (exact small-int arithmetic in f32) ----
            idsT = kp.tile([128, K], F32)
            for a in range(2):
                tp = ps.tile([128, 128], F32, tag="tp")
                nc.tensor.transpose(tp[:], ids_col[a][:].to_broadcast([128, 128]), ident[:])
                nc.vector.tensor_copy(idsT[:, a * 128:(a + 1) * 128], tp[:])

            LT, F_col = [], []
            for a in range(2):
                lt = kp.tile([128, K], F32, tag=f"lt{a}")
                nc.vector.tensor_tensor(lt[:], ids_col[a][:].to_broadcast([128, K]), idsT[:],
                                        op=ALU.is_lt)
                eq = wp.tile([128, K], F32, tag="eq")
                nc.vector.tensor_tensor(eq[:], ids_col[a][:].to_broadcast([128, K]), idsT[:],
                                        op=ALU.is_equal)
                nc.vector.tensor_mul(eq[:], eq[:], iltr[a][:])
                ec = wp.tile([128, 1], F32, tag="ec")
                nc.vector.tensor_reduce(out=ec[:], in_=eq[:],
                                        axis=mybir.AxisListType.X, op=ALU.add)
                fc = kp.tile([128, 1], F32, tag=f"fc{a}")
                nc.vector.tensor_scalar(fc[:], ec[:], 0.0, None, op0=ALU.is_equal)
                LT.append(lt)
                F_col.append(fc)

            pos_ps = psa.tile([1, K], F32, tag="acc")
            for a in range(2):
                nc.tensor.matmul(pos_ps[:], lhsT=F_col[a][:], rhs=LT[a][:],
                                 start=(a == 0), stop=(a == 1))
            pos = kp.tile([1, K], F32)
            nc.vector.tensor_copy(pos[:], pos_ps[:])

            one_cell = ones_row[0:1, 0:1]
            OH, RHS, OH16, RHS16 = [], [], [], []
            for a in range(2):
                pcp = ps.tile([128, 1], F32, tag="small")
                nc.tensor.matmul(pcp[:], lhsT=pos[0:1, a * 128:(a + 1) * 128], rhs=one_cell,
                                 start=True, stop=True)
                # q' = pos (0-based over the 256 non-cls slots) for survivors, DUMP otherwise
                nsv = wp.tile([128, 1], F32, tag="nsv")
                nc.vector.tensor_scalar(nsv[:], F_col[a][:], 0.5, DUMP, op0=ALU.is_lt, op1=ALU.mult)
                p1 = wp.tile([128, 1], F32, tag="p1")
                nc.vector.tensor_scalar(p1[:], pcp[:], F_col[a][:, 0:1], None, op0=ALU.mult)
                nc.vector.tensor_add(p1[:], p1[:], nsv[:])
                # (x = q' mod 128, c = q' div 128) for the [128, 2] output scatter
                d1 = wp.tile([128, 1], F32, tag="d1")
                nc.vector.tensor_scalar(d1[:], p1[:], 128.0, None, op0=ALU.is_ge)
                d2 = wp.tile([128, 1], F32, tag="d2")
                nc.vector.tensor_scalar(d2[:], p1[:], 256.0, None, op0=ALU.is_ge)
                dvs = wp.tile([128, 1], F32, tag="dvs")
                nc.vector.tensor_add(dvs[:], d1[:], d2[:])
                nc.vector.tensor_scalar(dvs[:], dvs[:], 128.0, None, op0=ALU.mult)
                md = wp.tile([128, 1], F32, tag="md")
                nc.vector.tensor_sub(md[:], p1[:], dvs[:])
                oh = kp.tile([128, 128], F32, tag=f"oh{a}")
                nc.vector.tensor_tensor(oh[:], md[:].to_broadcast([128, 128]), iota128[:],
                                        op=ALU.is_equal)
                dv0 = wp.tile([128, 1], F32, tag="dv0")
                nc.vector.tensor_scalar(dv0[:], p1[:], 128.0, None, op0=ALU.is_lt)
                dv1 = wp.tile([128, 1], F32, tag="dv1")
                nc.vector.tensor_sub(dv1[:], d1[:], d2[:])
                rhs = kp.tile([128, 2], F32, tag=f"rhs{a}")
                nc.vector.tensor_mul(rhs[:, 0:1], ids_col[a][:], dv0[:])
                nc.vector.tensor_mul(rhs[:, 1:2], ids_col[a][:], dv1[:])
                OH.append(oh)
                RHS.append(rhs)
                # chunk wrap: element q' -> chunk c = q'//128, local j = md = q'%128,
                # wrap-row = md%16, per-head col = 8c + md//16
                tdot = wp.tile([128, 128], F32, tag="tdot")
                m16x = wp.tile([128, 1], F32, tag="m16x")
                nc.vector.tensor_mul(tdot[:], oh[:], mod16t[:])
                nc.vector.tensor_reduce(out=m16x[:], in_=tdot[:],
                                        axis=mybir.AxisListType.X, op=ALU.add)
                s16h = wp.tile([128, 1], F32, tag="s16h")
                nc.vector.tensor_mul(tdot[:], oh[:], div16t[:])
                nc.vector.tensor_reduce(out=s16h[:], in_=tdot[:],
                                        axis=mybir.AxisListType.X, op=ALU.add)
                colw = wp.tile([128, 1], F32, tag="colw")
                nc.vector.tensor_scalar(colw[:], d1[:], 8.0, s16h[:, 0:1],
                                        op0=ALU.mult, op1=ALU.add)
                oh16 = kp.tile([128, 128], F32, tag=f"oh16{a}")
                nc.vector.tensor_tensor(oh16[:], m16x[:].to_broadcast([128, 128]), iota16r[:],
                                        op=ALU.is_equal)
                ohf = wp.tile([128, 16], F32, tag="ohf")
                nc.vector.tensor_tensor(ohf[:], colw[:].to_broadcast([128, 16]), iota16f[:],
                                        op=ALU.is_equal)
                vF = wp.tile([128, 1], F32, tag="vF")
                nc.vector.tensor_mul(vF[:], ids_col[a][:], F_col[a][:])
                rhs16 = kp.tile([128, 16], F32, tag=f"rhs16{a}")
                nc.vector.tensor_scalar(rhs16[:], ohf[:], vF[:, 0:1], None, op0=ALU.mult)
                OH16.append(oh16)
                RHS16.append(rhs16)

            vals_ps = psa.tile([128, 2], F32, tag="acc")
            for a in range(2):
                nc.tensor.matmul(vals_ps[:], lhsT=OH[a][:], rhs=RHS[a][:],
                                 start=(a == 0), stop=(a == 1))
            vals = kp.tile([128, 2], F32)  # (x, c) -> unique_ids[1 + 128c + x]
            nc.vector.tensor_copy(vals[:], vals_ps[:])

            v16_ps = psa.tile([128, 16], F32, tag="acc")
            for a in range(2):
                nc.tensor.matmul(v16_ps[:], lhsT=OH16[a][:], rhs=RHS16[a][:],
                                 start=(a == 0), stop=(a == 1))
            # wrapped ids: row p, col f -> unique_ids[1 + 16f + (p%16)], replicated 8x down
            idxw = kp.tile([128, 16 * H], F32)
            v16b = bass.AP(v16_ps[:].tensor, v16_ps[:].offset,
                           [v16_ps[:].ap[0], [0, H], v16_ps[:].ap[1]])
            nc.vector.tensor_tensor(idxw[:].rearrange("p (h f) -> p h f", f=16),
                                    v16b, hoffw[:].rearrange("p (h f) -> p h f", f=16),
                                    op=ALU.add)
            idx16 = kp.tile([128, 16 * H], I16)
            nc.vector.tensor_copy(idx16[:], idxw[:])

            # ---- stage 4: ids / mask outputs (row-transposed: few fat descriptors) ----
            vtp = ps.tile([2, 128], F32, tag="vtp")
            nc.tensor.transpose(vtp[:], vals[:], ident[:])
            vrow = kp.tile([2, 128], F32)
            nc.vector.tensor_copy(vrow[:], vtp[:])
            idsr = kp.tile([2, 128], I32)
            nc.vector.tensor_copy(idsr[:], vrow[:])
            nc.sync.dma_start(out=oids_d[:][0:1, None], in_=zero_i[:])
            nc.sync.dma_start(out=omask_d[:][0:1, None], in_=one_u8[:])
            nc.sync.dma_start(out=oids_d[:][None, 1:129], in_=idsr[0:1, :])
            nc.sync.dma_start(out=oids_d[:][None, 129:257], in_=idsr[1:2, :])
            mrow = kp.tile([2, 128], U8)
            nc.vector.tensor_scalar(mrow[:], vrow[:], 0.0, None, op0=ALU.not_equal)
            nc.sync.dma_start(out=omask_d[:][None, 1:129], in_=mrow[0:1, :])
            nc.sync.dma_start(out=omask_d[:][None, 129:257], in_=mrow[1:2, :])

            # ---- stage 5: gather attn rows (24 x 128 rows over 4 SWDGE queues) ----
            wengs = (nc.sync, nc.scalar)
            i = 0
            for h in range(H):
                for c in range(2):
                    g = gp.tile([128, W], F32, tag="g")
                    nc.gpsimd.dma_gather(
                        out_ap=g[:].rearrange("p (t f) -> p t f", f=W),
                        in_ap=attn_d[:],
                        idxs_ap=idx16[:, 16 * h + 8 * c:16 * h + 8 * c + 8],
                        num_idxs=128, num_idxs_reg=128, elem_size=W, queue_num=i % 4,
                    )
                    wengs[i % 2].dma_start(
                        out=oattn_d[:][h, 1 + c * 128:1 + (c + 1) * 128, :], in_=g[:],
                        single_packet=True)
                    i += 1

    nc.finalize()
    return nc


_NC = None


def _get_nc():
    global _NC
    if _NC is None:
        _NC = _build()
    return _NC


def _run(attn, value, mask, gumbel, trace=False):
    attn = np.asarray(attn, dtype=np.float32)
    value = np.ascontiguousarray(np.asarray(value, dtype=np.float32))
    gumbel = np.ascontiguousarray(np.asarray(gumbel, dtype=np.float32))
    mask_u8 = np.ascontiguousarray(np.asarray(mask).astype(np.uint8))

    attn_pad = np.zeros((B, NH, W), dtype=np.float32)
    attn_pad[:, :, :N] = attn.reshape(B, NH, N)

    # value relayout: token-partition-major so the device load is one fat DMA
    # [b, h, 1+8p+c, d] -> [b, p, (h, 8c+d...)]: partition p holds tokens 8p..8p+8 per head
    val_t = np.ascontiguousarray(
        value[:, :, 1:, :].reshape(B, H, 128, 512).transpose(0, 2, 1, 3).reshape(B, 128, H * 512))
    # cls scores input: attn[:, :, 0, 1:] in the same token-partition layout [p, (h, c)]
    cls_t = np.ascontiguousarray(
        attn[:, :, 0, 1:].reshape(B, H, 128, 8).transpose(0, 2, 1, 3).reshape(B, 128, H * 8))
    in_maps = [
        {
            "attn": attn_pad[b],
            "value": val_t[b],
            "clsp": cls_t[b],
            "maskp": mask_u8[b],
            "gumbel": gumbel[b],
        }
        for b in range(B)
    ]
    nc = _get_nc()
    res = run_bass_kernel_spmd(nc, in_maps, list(range(B)), trace=trace)

    new_attn = np.ascontiguousarray(
        np.stack([np.asarray(res.results[b]["out_attn"]) for b in range(B)])[:, :, :, :N])
    unique_ids = np.stack([np.asarray(res.results[b]["out_ids"]) for b in range(B)])
    new_mask = np.stack([np.asarray(res.results[b]["out_mask"]) for b in range(B)]).astype(bool)
    return (new_attn, new_mask, unique_ids.astype(np.int32)), res


def kernel(attn, value, mask, gumbel):
    out, _ = _run(attn, value, mask, gumbel, trace=False)
    return out


# revision 25
# speedup vs baseline: 1.1044x; 1.1044x over previous
"""AdaptiveTokenSampling on 8 TRN2 NeuronCores (Bass/Tile, batch-parallel).

Per-core (one batch element):
  1. score pipeline: value norms + cls attention -> pseudo-logits (token-partition layout)
  2. gumbel argmax sampling (vector.max/max_index) -> 256 sampled token ids
  3. sort-based unique via comparison matrices (DVE) + positional scatter (PE matmuls)
  4. row gather of attn via dma_gather across 4 SWDGE queues; writes on both HWDGE queues

The attn input is host-padded to 1088-float rows (4352 B, a multiple of 256) so the
custom-ISA dma_gather row stride constraint holds.
"""
import numpy as np

import concourse.bacc as bacc
import concourse.bass as bass
import concourse.mybir as mybir
import concourse.tile as tile
from concourse.bass_utils import run_bass_kernel_spmd

F32 = mybir.dt.float32
I32 = mybir.dt.int32
I16 = mybir.dt.int16
U32 = mybir.dt.uint32
U8 = mybir.dt.uint8

B, H, N, D, K = 8, 12, 1025, 64, 256
W = 1088             # padded attn row length (f32), 4352 B stride for dma_gather
NM1 = N - 1          # 1024
NH = N * H           # 12300 rows in padded attn table
KP1 = K + 1          # 257
EPS = 1e-6
MASKVAL = float(-np.finfo(np.float32).max / 2)
DUMP = 300.0         # parking slot for non-survivors (contributes 0 everywhere)
VCH = 2              # heads per value-pipeline chunk
ALU = mybir.AluOpType


def _build():
    nc = bacc.Bacc(None, target_bir_lowering=False, debug=False, num_devices=8,
                   num_swdge_queues=4)

    attn_d = nc.declare_dram_parameter("attn", [NH, W], F32, isOutput=False)
    val_d = nc.declare_dram_parameter("value", [128, H * 512], F32, isOutput=False)
    cls_d = nc.declare_dram_parameter("clsp", [128, H * 8], F32, isOutput=False)
    msk_d = nc.declare_dram_parameter("maskp", [N], U8, isOutput=False)
    gum_d = nc.declare_dram_parameter("gumbel", [K, NM1], F32, isOutput=False)

    oattn_d = nc.declare_dram_parameter("out_attn", [H, KP1, W], F32, isOutput=True)
    oids_d = nc.declare_dram_parameter("out_ids", [KP1], I32, isOutput=True)
    omask_d = nc.declare_dram_parameter("out_mask", [KP1], U8, isOutput=True)

    pl_dram = nc.dram_tensor("pl_dram", [NM1], F32)

    ident_c = nc.inline_tensor(np.eye(128, dtype=np.float32), name="ident_c")
    iota128_c = nc.inline_tensor(
        np.broadcast_to(np.arange(128, dtype=np.float32), (128, 128)).copy(), name="iota128_c")
    iota16r_c = nc.inline_tensor(
        np.broadcast_to(np.arange(128, dtype=np.float32) % 16, (128, 128)).copy(),
        name="iota16r_c")  # [p, x] = x % 16 (same every row)
    iota16f_c = nc.inline_tensor(
        np.broadcast_to(np.arange(16, dtype=np.float32), (128, 16)).copy(), name="iota16f_c")
    x128 = np.arange(128, dtype=np.float32)
    mod16_c = nc.inline_tensor(np.broadcast_to(x128 % 16, (128, 128)).copy(), name="mod16_c")
    div16_c = nc.inline_tensor(np.broadcast_to(x128 // 16, (128, 128)).copy(), name="div16_c")
    gidx = np.arange(K, dtype=np.float32)
    p128 = np.arange(128, dtype=np.float32)
    # ILTrev_a[p, q] = 1.0 if q < (p+128a): count of earlier-equal, target on partition
    iltr_c = [nc.inline_tensor((gidx[None, :] < (p128 + 128 * a)[:, None]).astype(np.float32),
                               name=f"iltr{a}_c") for a in range(2)]
    # head offsets for wrapped gather indices: [128, 16h+f] -> h*N
    hoffw_c = nc.inline_tensor(
        np.broadcast_to(np.repeat(np.arange(H, dtype=np.float32) * float(N), 16), (128, 16 * H)).copy(),
        name="hoffw_c")
    dummyi_c = nc.inline_tensor(np.zeros((128, 1), dtype=np.int16), name="dummyi_c")

    with tile.TileContext(nc) as tc:
        with (
            tc.tile_pool(name="const", bufs=1) as cp,
            tc.tile_pool(name="work", bufs=2) as wp,
            tc.tile_pool(name="keep", bufs=1) as kp,
            tc.tile_pool(name="ps", bufs=2, space="PSUM") as ps,
            tc.tile_pool(name="psacc", bufs=2, space="PSUM") as psa,
            tc.tile_pool(name="gath", bufs=16) as gp,
        ):
            # ---- tiny DVE constants + ACT table warmups (off critical path) ----
            ones_col = cp.tile([128, 1], F32)
            nc.vector.memset(ones_col[:], 1.0)
            ones_row = cp.tile([1, 128], F32)
            nc.vector.memset(ones_row[:], 1.0)
            eps_col = cp.tile([128, 1], F32)
            nc.vector.memset(eps_col[:], EPS)
            zero_i = cp.tile([1, 1], I32)
            nc.vector.memset(zero_i[:], 0)
            one_u8 = cp.tile([1, 1], U8)
            nc.vector.memset(one_u8[:], 1)
            dumm = wp.tile([1, 1], F32, tag="dumm")
            nc.scalar.square(dumm[:], ones_col[0:1, 0:1])
            nc.scalar.activation(dumm[:], ones_col[0:1, 0:1], mybir.ActivationFunctionType.Sqrt)
            nc.scalar.activation(dumm[:], ones_col[0:1, 0:1], mybir.ActivationFunctionType.Ln,
                                 bias=eps_col[0:1, 0:1], scale=1.0)

            # ---- gpsimd library warmup: tiny dma_gather so the Q7 IRAM load happens early ----
            dzi = cp.tile([128, 1], I16)
            nc.scalar.dma_start(out=dzi[:], in_=dummyi_c[:])
            dg = gp.tile([128, W], F32, tag="g")
            nc.gpsimd.dma_gather(
                out_ap=dg[:].rearrange("p (t f) -> p t f", f=W),
                in_ap=attn_d[:], idxs_ap=dzi[:], num_idxs=16, num_idxs_reg=16,
                elem_size=W, queue_num=0,
            )


            # ---- input loads first: value chunks (both queues) + gumbel ----
            vts = []
            for k in range(H // VCH):
                vt = kp.tile([128, VCH * 512], F32, tag=f"vt{k}")
                (nc.sync if k % 2 == 0 else nc.scalar).dma_start(
                    out=vt[:], in_=val_d[:][:, k * VCH * 512:(k + 1) * VCH * 512])
                vts.append(vt)
            gts = []
            for a in range(2):
                gt = kp.tile([128, NM1], F32, tag=f"gt{a}")
                (nc.sync if a == 0 else nc.scalar).dma_start(
                    out=gt[:], in_=gum_d[:][a * 128:(a + 1) * 128, :])
                gts.append(gt)

            # ---- static work with no deps: cls row of new_attn, ids[0], mask[0] ----
            g0 = cp.tile([H, W], F32, tag="g0")
            nc.scalar.dma_start(out=g0[:], in_=bass.AP(attn_d, 0, [[N * W, H], [1, W]]))
            nc.scalar.dma_start(out=oattn_d[:][:, 0, :], in_=g0[:], single_packet=True)

            # ---- constants (scalar queue; value pipeline owns the sync queue) ----
            ident = cp.tile([128, 128], F32)
            nc.scalar.dma_start(out=ident[:], in_=ident_c[:])
            iota128 = cp.tile([128, 128], F32)
            nc.scalar.dma_start(out=iota128[:], in_=iota128_c[:])
            iota16r = cp.tile([128, 128], F32)
            nc.scalar.dma_start(out=iota16r[:], in_=iota16r_c[:])
            iota16f = cp.tile([128, 16], F32)
            nc.scalar.dma_start(out=iota16f[:], in_=iota16f_c[:])
            mod16t = cp.tile([128, 128], F32)
            nc.scalar.dma_start(out=mod16t[:], in_=mod16_c[:])
            div16t = cp.tile([128, 128], F32)
            nc.scalar.dma_start(out=div16t[:], in_=div16_c[:])
            iltr = []
            for a in range(2):
                t2 = cp.tile([128, K], F32, tag=f"iltr{a}")
                nc.scalar.dma_start(out=t2[:], in_=iltr_c[a][:])
                iltr.append(t2)
            hoffw = cp.tile([128, 16 * H], F32)
            nc.scalar.dma_start(out=hoffw[:], in_=hoffw_c[:])
            cls = kp.tile([128, H * 8], F32)
            nc.gpsimd.dma_start(out=cls[:], in_=cls_d[:])
            mku = wp.tile([128, 8], U8)
            nc.gpsimd.dma_start(out=mku[:], in_=msk_d[:][None, 1:].rearrange("o (p c) -> (o p) c", c=8))

            # PE warmup: observe const DMAs once so PE-transposes need only a DVE wait
            warm = ps.tile([1, 1], F32, tag="small")
            nc.tensor.matmul(warm[:], lhsT=ident[:, 0:1], rhs=iota128[:, 0:1],
                             start=True, stop=True)

            # ---- stage 1: scores (token-partition layout: token j-1 = 8p + c) ----
            norms2 = kp.tile([128, H * 8], F32)
            for k in range(H // VCH):
                h0 = k * VCH
                sqc = wp.tile([128, VCH * 512], F32, tag="sqc")
                nc.scalar.square(sqc[:], vts[k][:])
                nc.vector.tensor_reduce(
                    out=norms2[:, h0 * 8:(h0 + VCH) * 8],
                    in_=sqc[:].rearrange("p (g d) -> p g d", d=D),
                    axis=mybir.AxisListType.X, op=ALU.add)
            norms = kp.tile([128, H * 8], F32)
            nc.scalar.sqrt(norms[:], norms2[:])

            prod = kp.tile([128, H * 8], F32)
            nc.vector.tensor_mul(prod[:], cls[:], norms[:])
            score = kp.tile([128, 8], F32)
            nc.vector.tensor_reduce(
                out=score[:], in_=prod[:].rearrange("p (h c) -> p c h", c=8),
                axis=mybir.AxisListType.X, op=ALU.add)

            sumrow = wp.tile([128, 1], F32)
            nc.vector.tensor_reduce(out=sumrow[:], in_=score[:],
                                    axis=mybir.AxisListType.X, op=ALU.add)
            total_ps = ps.tile([1, 1], F32, tag="small")
            nc.tensor.matmul(total_ps[:], lhsT=sumrow[:], rhs=ones_col[:],
                             start=True, stop=True)
            total = wp.tile([1, 1], F32)
            nc.vector.tensor_scalar(total[:], total_ps[:], EPS, None, op0=ALU.add)
            recip = wp.tile([1, 1], F32)
            nc.vector.reciprocal(recip[:], total[:])
            # broadcast recip to 128 partitions: K=1 matmul with a ones row (exact: 1.0*x)
            recB_ps = ps.tile([128, 1], F32, tag="small")
            nc.tensor.matmul(recB_ps[:], lhsT=ones_row[:], rhs=recip[:],
                             start=True, stop=True)
            recipB = wp.tile([128, 1], F32)
            nc.vector.tensor_copy(recipB[:], recB_ps[:])

            pl = kp.tile([128, 8], F32)
            nc.scalar.activation(pl[:], score[:], mybir.ActivationFunctionType.Ln,
                                 bias=eps_col[:, 0:1], scale=recipB[:, 0:1])
            # mask (all ones in practice; exact reference semantics)
            mkf = wp.tile([128, 8], F32)
            nc.vector.tensor_copy(mkf[:], mku[:])
            plm = kp.tile([128, 8], F32)
            nc.vector.tensor_mul(plm[:], pl[:], mkf[:])
            inv = wp.tile([128, 8], F32)
            nc.vector.tensor_scalar(inv[:], mkf[:], 0.5, MASKVAL, op0=ALU.is_lt, op1=ALU.mult)
            nc.vector.tensor_add(plm[:], plm[:], inv[:])
            nc.sync.dma_start(out=pl_dram[:].rearrange("(p c) -> p c", c=8), in_=plm[:])
            # replicate pl to all 128 partitions (exact bytes via DRAM replicate read)
            plB = kp.tile([128, NM1], F32)
            nc.sync.dma_start(out=plB[:], in_=bass.AP(pl_dram, 0, [[0, 128], [1, NM1]]))

            # ---- stage 2: gumbel argmax sampling ----
            ids_col = []
            for a in range(2):
                lg = wp.tile([128, NM1], F32, tag="lg")
                nc.vector.tensor_add(lg[:], gts[a][:], plB[:])
                mx8 = wp.tile([128, 8], F32, tag="mx8")
                nc.vector.max(out=mx8[:], in_=lg[:])
                ix8 = wp.tile([128, 8], U32, tag="ix8")
                nc.vector.max_index(out=ix8[:], in_max=mx8[:], in_values=lg[:])
                idc = kp.tile([128, 1], F32, tag=f"idc{a}")
                nc.vector.tensor_scalar(idc[:], ix8[:, 0:1], 1.0, None, op0=ALU.add)
                ids_col.append(idc)

            # ---- stage 3: unique + sorted positions (exact small-int arithmetic in f32) ----
            idsT = kp.tile([128, K], F32)
            for a in range(2):
                tp = ps.tile([128, 128], F32, tag="tp")
                nc.tensor.transpose(tp[:], ids_col[a][:].to_broadcast([128, 128]), ident[:])
                nc.vector.tensor_copy(idsT[:, a * 128:(a + 1) * 128], tp[:])

            LT, F_col = [], []
            for a in range(2):
                lt = kp.tile([128, K], F32, tag=f"lt{a}")
                nc.vector.tensor_tensor(lt[:], ids_col[a][:].to_broadcast([128, K]), idsT[:],
                                        op=ALU.is_lt)
                eq = wp.tile([128, K], F32, tag="eq")
                nc.vector.tensor_tensor(eq[:], ids_col[a][:].to_broadcast([128, K]), idsT[:],
                                        op=ALU.is_equal)
                nc.vector.tensor_mul(eq[:], eq[:], iltr[a][:])
                ec = wp.tile([128, 1], F32, tag="ec")
                nc.vector.tensor_reduce(out=ec[:], in_=eq[:],
                                        axis=mybir.AxisListType.X, op=ALU.add)
                fc = kp.tile([128, 1], F32, tag=f"fc{a}")
                nc.vector.tensor_scalar(fc[:], ec[:], 0.0, None, op0=ALU.is_equal)
                LT.append(lt)
                F_col.append(fc)

            pos_ps = psa.tile([1, K], F32, tag="acc")
            for a in range(2):
                nc.tensor.matmul(pos_ps[:], lhsT=F_col[a][:], rhs=LT[a][:],
                                 start=(a == 0), stop=(a == 1))
            pos = kp.tile([1, K], F32)
            nc.vector.tensor_copy(pos[:], pos_ps[:])

            one_cell = ones_row[0:1, 0:1]
            OH, RHS, OH16, RHS16 = [], [], [], []
            for a in range(2):
                pcp = ps.tile([128, 1], F32, tag="small")
                nc.tensor.matmul(pcp[:], lhsT=pos[0:1, a * 128:(a + 1) * 128], rhs=one_cell,
                                 start=True, stop=True)
                # q' = pos (0-based over the 256 non-cls slots) for survivors, DUMP otherwise
                nsv = wp.tile([128, 1], F32, tag="nsv")
                nc.vector.tensor_scalar(nsv[:], F_col[a][:], 0.5, DUMP, op0=ALU.is_lt, op1=ALU.mult)
                p1 = wp.tile([128, 1], F32, tag="p1")
                nc.vector.tensor_scalar(p1[:], pcp[:], F_col[a][:, 0:1], None, op0=ALU.mult)
                nc.vector.tensor_add(p1[:], p1[:], nsv[:])
                # (x = q' mod 128, c = q' div 128) for the [128, 2] output scatter
                d1 = wp.tile([128, 1], F32, tag="d1")
                nc.vector.tensor_scalar(d1[:], p1[:], 128.0, None, op0=ALU.is_ge)
                d2 = wp.tile([128, 1], F32, tag="d2")
                nc.vector.tensor_scalar(d2[:], p1[:], 256.0, None, op0=ALU.is_ge)
                dvs = wp.tile([128, 1], F32, tag="dvs")
                nc.vector.tensor_add(dvs[:], d1[:], d2[:])
                nc.vector.tensor_scalar(dvs[:], dvs[:], 128.0, None, op0=ALU.mult)
                md = wp.tile([128, 1], F32, tag="md")
                nc.vector.tensor_sub(md[:], p1[:], dvs[:])
                oh = kp.tile([128, 128], F32, tag=f"oh{a}")
                nc.vector.tensor_tensor(oh[:], md[:].to_broadcast([128, 128]), iota128[:],
                                        op=ALU.is_equal)
                dv0 = wp.tile([128, 1], F32, tag="dv0")
                nc.vector.tensor_scalar(dv0[:], p1[:], 128.0, None, op0=ALU.is_lt)
                dv1 = wp.tile([128, 1], F32, tag="dv1")
                nc.vector.tensor_sub(dv1[:], d1[:], d2[:])
                rhs = kp.tile([128, 2], F32, tag=f"rhs{a}")
                nc.vector.tensor_mul(rhs[:, 0:1], ids_col[a][:], dv0[:])
                nc.vector.tensor_mul(rhs[:, 1:2], ids_col[a][:], dv1[:])
                OH.append(oh)
                RHS.append(rhs)
                # chunk wrap: element q' -> chunk c = q'//128, local j = md = q'%128,
                # wrap-row = md%16, per-head col = 8c + md//16
                tdot = wp.tile([128, 128], F32, tag="tdot")
                m16x = wp.tile([128, 1], F32, tag="m16x")
                nc.vector.tensor_mul(tdot[:], oh[:], mod16t[:])
                nc.vector.tensor_reduce(out=m16x[:], in_=tdot[:],
                                        axis=mybir.AxisListType.X, op=ALU.add)
                s16h = wp.tile([128, 1], F32, tag="s16h")
                nc.vector.tensor_mul(tdot[:], oh[:], div16t[:])
                nc.vector.tensor_reduce(out=s16h[:], in_=tdot[:],
                                        axis=mybir.AxisListType.X, op=ALU.add)
                colw = wp.tile([128, 1], F32, tag="colw")
                nc.vector.tensor_scalar(colw[:], d1[:], 8.0, s16h[:, 0:1],
                                        op0=ALU.mult, op1=ALU.add)
                oh16 = kp.tile([128, 128], F32, tag=f"oh16{a}")
                nc.vector.tensor_tensor(oh16[:], m16x[:].to_broadcast([128, 128]), iota16r[:],
                                        op=ALU.is_equal)
                ohf = wp.tile([128, 16], F32, tag="ohf")
                nc.vector.tensor_tensor(ohf[:], colw[:].to_broadcast([128, 16]), iota16f[:],
                                        op=ALU.is_equal)
                vF = wp.tile([128, 1], F32, tag="vF")
                nc.vector.tensor_mul(vF[:], ids_col[a][:], F_col[a][:])
                rhs16 = kp.tile([128, 16], F32, tag=f"rhs16{a}")
                nc.vector.tensor_scalar(rhs16[:], ohf[:], vF[:, 0:1], None, op0=ALU.mult)
                OH16.append(oh16)
                RHS16.append(rhs16)

            vals_ps = psa.tile([128, 2], F32, tag="acc")
            for a in range(2):
                nc.tensor.matmul(vals_ps[:], lhsT=OH[a][:], rhs=RHS[a][:],
                                 start=(a == 0), stop=(a == 1))
            vals = kp.tile([128, 2], F32)  # (x, c) -> unique_ids[1 + 128c + x]
            nc.vector.tensor_copy(vals[:], vals_ps[:])

            v16_ps = psa.tile([128, 16], F32, tag="acc")
            for a in range(2):
                nc.tensor.matmul(v16_ps[:], lhsT=OH16[a][:], rhs=RHS16[a][:],
                                 start=(a == 0), stop=(a == 1))
            # wrapped ids: row p, col f -> unique_ids[1 + 16f + (p%16)], replicated 8x down
            idxw = kp.tile([128, 16 * H], F32)
            v16b = bass.AP(v16_ps[:].tensor, v16_ps[:].offset,
                           [v16_ps[:].ap[0], [0, H], v16_ps[:].ap[1]])
            nc.vector.tensor_tensor(idxw[:].rearrange("p (h f) -> p h f", f=16),
                                    v16b, hoffw[:].rearrange("p (h f) -> p h f", f=16),
                                    op=ALU.add)
            idx16 = kp.tile([128, 16 * H], I16)
            nc.vector.tensor_copy(idx16[:], idxw[:])

            # ---- stage 4: ids / mask outputs (row-transposed: few fat descriptors) ----
            vtp = ps.tile([2, 128], F32, tag="vtp")
            nc.tensor.transpose(vtp[:], vals[:], ident[:])
            vrow = kp.tile([2, 128], F32)
            nc.vector.tensor_copy(vrow[:], vtp[:])
            idsr = kp.tile([2, 128], I32)
            nc.vector.tensor_copy(idsr[:], vrow[:])
            nc.sync.dma_start(out=oids_d[:][0:1, None], in_=zero_i[:])
            nc.sync.dma_start(out=omask_d[:][0:1, None], in_=one_u8[:])
            nc.sync.dma_start(out=oids_d[:][None, 1:129], in_=idsr[0:1, :])
            nc.sync.dma_start(out=oids_d[:][None, 129:257], in_=idsr[1:2, :])
            mrow = kp.tile([2, 128], U8)
            nc.vector.tensor_scalar(mrow[:], vrow[:], 0.0, None, op0=ALU.not_equal)
            nc.sync.dma_start(out=omask_d[:][None, 1:129], in_=mrow[0:1, :])
            nc.sync.dma_start(out=omask_d[:][None, 129:257], in_=mrow[1:2, :])

            # ---- stage 5: gather attn rows (24 x 128 rows over 4 SWDGE queues) ----
            wengs = (nc.sync, nc.scalar)
            i = 0
            for h in range(H):
                for c in range(2):
                    g = gp.tile([128, W], F32, tag="g")
                    nc.gpsimd.dma_gather(
                        out_ap=g[:].rearrange("p (t f) -> p t f", f=W),
                        in_ap=attn_d[:],
                        idxs_ap=idx16[:, 16 * h + 8 * c:16 * h + 8 * c + 8],
                        num_idxs=128, num_idxs_reg=128, elem_size=W, queue_num=i % 4,
                    )
                    wengs[i % 2].dma_start(
                        out=oattn_d[:][h, 1 + c * 128:1 + (c + 1) * 128, :], in_=g[:],
                        single_packet=True)
                    i += 1

    nc.finalize()
    return nc


_NC = None


def _get_nc():
    global _NC
    if _NC is None:
        _NC = _build()
    return _NC


def _run(attn, value, mask, gumbel, trace=False):
    attn = np.asarray(attn, dtype=np.float32)
    value = np.ascontiguousarray(np.asarray(value, dtype=np.float32))
    gumbel = np.ascontiguousarray(np.asarray(gumbel, dtype=np.float32))
    mask_u8 = np.ascontiguousarray(np.asarray(mask).astype(np.uint8))

    attn_pad = np.zeros((B, NH, W), dtype=np.float32)
    attn_pad[:, :, :N] = attn.reshape(B, NH, N)

    # value relayout: token-partition-major so the device load is one fat DMA
    # [b, h, 1+8p+c, d] -> [b, p, (h, 8c+d...)]: partition p holds tokens 8p..8p+8 per head
    val_t = np.ascontiguousarray(
        value[:, :, 1:, :].reshape(B, H, 128, 512).transpose(0, 2, 1, 3).reshape(B, 128, H * 512))
    # cls scores input: attn[:, :, 0, 1:] in the same token-partition layout [p, (h, c)]
    cls_t = np.ascontiguousarray(
        attn[:, :, 0, 1:].reshape(B, H, 128, 8).transpose(0, 2, 1, 3).reshape(B, 128, H * 8))
    in_maps = [
        {
            "attn": attn_pad[b],
            "value": val_t[b],
            "clsp": cls_t[b],
            "maskp": mask_u8[b],
            "gumbel": gumbel[b],
        }
        for b in range(B)
    ]
    nc = _get_nc()
    res = run_bass_kernel_spmd(nc, in_maps, list(range(B)), trace=trace)

    new_attn = np.ascontiguousarray(
        np.stack([np.asarray(res.results[b]["out_attn"]) for b in range(B)])[:, :, :, :N])
    unique_ids = np.stack([np.asarray(res.results[b]["out_ids"]) for b in range(B)])
    new_mask = np.stack([np.asarray(res.results[b]["out_mask"]) for b in range(B)]).astype(bool)
    return (new_attn, new_mask, unique_ids.astype(np.int32)), res


def kernel(attn, value, mask, gumbel):
    out, _ = _run(attn, value, mask, gumbel, trace=False)
    return out
